# revision 34
# baseline (speedup 1.0000x reference)
"""Trainium2 Bass kernel for nn_AdaptiveAdjacencyMatrix.

Reference math:
    s[b, i]        = sum_d h[b, i, d] * w[d]
    scores[b,i,j]  = s[b,i] + s[b,j] + bias
    A              = softmax(scores, axis=1)   # over i

Because the softmax is over axis=1 (i), the `s[b,j] + bias` term is constant
along the reduced axis and cancels exactly:
    A[b, i, j] = exp(s[b,i]) / sum_i' exp(s[b,i'])   (independent of j and bias)

So the output is a column-broadcast of softmax(s[b]) — the kernel is purely
memory-bound: write 4*4096*4096*4 = 268 MB of output at HBM speed.

Sharding: 8 cores = (batch b, row-half rh). Each core receives the full
h[b] (rows reordered so its own 2048 rows come first), computes softmax(s)
locally (the softmax sum needs all 4096 rows anyway; row order is
irrelevant to the sum), and writes a [2048, 4096] output shard. No
collectives needed.

Layouts: h is DMA'd with fully-contiguous per-partition descriptors
(partition p holds rows 16p..16p+15 of its half), which makes the on-chip
softmax come out in a (q, r) layout where device output row 128r + q holds
the value for local input row 16q + r. The host unshard undoes that with a
cheap reshape/transpose.
"""

import ml_dtypes
import numpy as np

B, N, D = 4, 4096, 256
NCORES = 8
HALF = N // 2          # 2048 rows written per core
P = 128                # SBUF partitions
RPP = HALF // P        # 16 rows per partition (per half)
DOT_CHUNK = 4          # rows-per-partition per h-load/dot chunk (512 KB)
SUP = 2                # groups per output supertile / DMA (4 MB)

_CACHE = {}


def _build():
    import concourse.mybir as mybir
    import concourse.tile as tile
    from concourse import bacc

    f32 = mybir.dt.float32
    Copy = mybir.ActivationFunctionType.Copy
    nc = bacc.Bacc("TRN2", target_bir_lowering=False, debug=False)

    bf16 = mybir.dt.bfloat16
    h_ext = nc.declare_dram_parameter("h", [N, D], bf16, isOutput=False)
    # w arrives pre-broadcast to 128 partitions (host-side tile of the 1 KB
    # vector) so the dot-product chain only waits on one small DMA.
    w_ext = nc.declare_dram_parameter("wb", [P, D], bf16, isOutput=False)
    out_ext = nc.declare_dram_parameter("out", [HALF, N], f32, isOutput=True)

    # contiguous flat views: partition p holds rows 16p..16p+15 of each half
    h_mine = h_ext[0:HALF, :].rearrange("(p r) d -> p r d", p=P)
    h_oth = h_ext[HALF:N, :].rearrange("(p r) d -> p r d", p=P)
    # [128, r, j] view of out: device out row = 128r + q (host un-permutes)
    out_r = out_ext[:, :].rearrange("(r q) j -> q r j", q=P)

    with tile.TileContext(nc) as tc:
        with (
            tc.tile_pool(name="const", bufs=1) as cpool,
            tc.tile_pool(name="hload", bufs=4) as hpool,
            tc.tile_pool(name="prod", bufs=4) as ppool,
            tc.tile_pool(name="small", bufs=1) as spool,
            tc.tile_pool(name="obuf", bufs=3) as opool,
            tc.tile_pool(name="psum", bufs=1, space="PSUM") as psum_pool,
        ):
            # all-ones [128,128] for the PE cross-partition-sum trick
            ones_k = cpool.tile([P, P], f32)
            nc.vector.memset(ones_k[:, :], 1.0)

            # --- w, pre-broadcast on host; first on the sync ring ---
            w_bc = cpool.tile([P, D], bf16)
            nc.sync.dma_start(out=w_bc[:, :], in_=w_ext[:, :])
            # materialized repeat (real strides, so DVE 2x bf16 mode applies)
            w_rep = cpool.tile([P, DOT_CHUNK, D], bf16)
            nc.vector.tensor_copy(
                w_rep[:, :, :],
                w_bc[:, :].unsqueeze(1).broadcast_to([P, DOT_CHUNK, D]),
            )

            # --- s = h @ w for both halves, [128, 16] each.
            # All h DMAs go on the sync ring only (FIFO: first chunk lands
            # fast; the scalar ring's DMAs would queue behind ACT compute).
            # DVE does all multiplies; reductions split ACT/DVE to balance
            # (ACT accum-reduce costs ~0.69us per row vs DVE 1.22us per
            # 4-row chunk). ---
            s_mine = spool.tile([P, RPP], f32)
            s_oth = spool.tile([P, RPP], f32)
            e_mine = spool.tile([P, RPP], f32)
            rs_m = spool.tile([P, 1], f32)
            rs_o = spool.tile([P, 1], f32)
            tot_psum = psum_pool.tile([P, 1], f32)
            jnk = spool.tile([P, D], f32)
            dve_reduce_chunks = {2, 4, 6, 7}
            n_chunks = RPP // DOT_CHUNK  # 4 per half
            for half in range(2):
                h_src = h_mine if half == 0 else h_oth
                s_dst = s_mine if half == 0 else s_oth
                for c in range(n_chunks):
                    ci = half * n_chunks + c
                    hch = hpool.tile([P, DOT_CHUNK, D], bf16)
                    nc.sync.dma_start(
                        out=hch[:, :, :],
                        in_=h_src[:, c * DOT_CHUNK : (c + 1) * DOT_CHUNK, :],
                    )
                    prod = ppool.tile([P, DOT_CHUNK, D], bf16)
                    nc.vector.tensor_tensor(
                        out=prod[:, :, :],
                        in0=hch[:, :, :],
                        in1=w_rep[:, :, :],
                        op=mybir.AluOpType.mult,
                    )
                    if ci not in dve_reduce_chunks:
                        for g in range(DOT_CHUNK):
                            gi = c * DOT_CHUNK + g
                            nc.scalar.activation(
                                out=jnk[:, :],
                                in_=prod[:, g, :],
                                func=Copy,
                                accum_out=s_dst[:, gi : gi + 1],
                            )
                    else:
                        nc.vector.tensor_reduce(
                            out=s_dst[:, c * DOT_CHUNK : (c + 1) * DOT_CHUNK],
                            in_=prod[:, :, :],
                            axis=mybir.AxisListType.X,
                            op=mybir.AluOpType.add,
                        )
                if half == 0:
                    # mine half done: exp + its partition-sum can overlap the
                    # other half's dot products
                    nc.scalar.activation(
                        out=e_mine[:, :],
                        in_=s_mine[:, :],
                        func=mybir.ActivationFunctionType.Exp,
                        accum_out=rs_m[:, 0:1],
                    )
                    # route rs through DVE so the PE matmul needs only one
                    # wait (its LdWeights slot fits a single semaphore)
                    rs_m2 = spool.tile([P, 1], f32)
                    nc.vector.tensor_copy(rs_m2[:, 0:1], rs_m[:, 0:1])
                    nc.tensor.matmul(
                        tot_psum[:, 0:1],
                        ones_k[:, 0:P],
                        rs_m2[:, 0:1],
                        start=True,
                        stop=False,
                    )

            # --- finish the softmax sum: exp(other), accumulate its
            # partition-sum into the same PSUM bank, reciprocal, scale ---
            e_oth = spool.tile([P, RPP], f32)
            nc.scalar.activation(
                out=e_oth[:, :],
                in_=s_oth[:, :],
                func=mybir.ActivationFunctionType.Exp,
                accum_out=rs_o[:, 0:1],
            )
            nc.tensor.matmul(
                tot_psum[:, 0:1], ones_k[:, 0:P], rs_o[:, 0:1], start=False, stop=True
            )
            inv = spool.tile([P, 1], f32)
            nc.vector.reciprocal(inv[:, 0:1], tot_psum[:, 0:1])

            # --- broadcast p along columns (stride-0 reads) and stream out ---
            for t in range(RPP // SUP):
                ot = opool.tile([P, SUP * N], f32)
                for g in range(SUP):
                    gi = t * SUP + g
                    # p = e * (1/S) folded into the broadcast multiply
                    col_b = e_mine[:, gi : gi + 1].broadcast_to([P, N])
                    dst = ot[:, g * N : (g + 1) * N]
                    nc.vector.tensor_scalar_mul(dst, col_b, inv[:, 0:1])
                # alternate the two HWDGE rings for more DMA queue depth
                out_dma_eng = nc.sync if t % 2 == 0 else nc.scalar
                out_dma_eng.dma_start(
                    out=out_r[:, t * SUP : (t + 1) * SUP, :],
                    in_=ot[:, :].rearrange("q (r j) -> q r j", r=SUP),
                )
    nc.compile()
    return nc


def _get_nc():
    if "nc" not in _CACHE:
        _CACHE["nc"] = _build()
    return _CACHE["nc"]


def _ensure_axon_hooks():
    """bass_utils' trace path imports antenv.axon_hooks, which some images
    lack; provide a no-op stub so tracing degrades instead of crashing."""
    try:
        import antenv.axon_hooks  # noqa: F401
    except ImportError:
        import sys
        import types

        try:
            import antenv
        except ImportError:
            antenv = types.ModuleType("antenv")
            sys.modules["antenv"] = antenv
        m = types.ModuleType("antenv.axon_hooks")
        m._hook = None
        m.set_axon_ntff_profile_hook = lambda h: setattr(m, "_hook", h)
        m.get_axon_ntff_profile_hook = lambda: m._hook
        sys.modules["antenv.axon_hooks"] = m


def run_on_device(h, w, trace=False):
    """Run the SPMD kernel; returns the BassKernelResults."""
    from concourse.bass_utils import run_bass_kernel_spmd

    _ensure_axon_hooks()

    in_maps = []
    for c in range(NCORES):
        b_idx, rh = divmod(c, 2)
        hb = h[b_idx]
        if rh:
            hb = np.concatenate([hb[HALF:], hb[:HALF]], axis=0)
        in_maps.append(
            {
                "h": np.ascontiguousarray(hb.astype(ml_dtypes.bfloat16)),
                "wb": np.ascontiguousarray(
                    np.broadcast_to(w.astype(ml_dtypes.bfloat16), (P, D))
                ),
            }
        )
    res = run_bass_kernel_spmd(
        _get_nc(), in_maps, core_ids=list(range(NCORES)), trace=trace
    )
    return res


def kernel(h, w, b):
    h = np.asarray(h, dtype=np.float32)
    w = np.asarray(w, dtype=np.float32)
    res = run_on_device(h, w)
    A = np.empty((B, N, N), dtype=np.float32)
    for c in range(NCORES):
        b_idx, rh = divmod(c, 2)
        out_c = res.results[c]["out"]
        # device row 128r + q holds the value for local input row 16q + r:
        # undo with reshape/transpose
        unperm = (
            out_c.reshape(RPP, P, N).transpose(1, 0, 2).reshape(HALF, N)
        )
        A[b_idx, rh * HALF : (rh + 1) * HALF, :] = unperm
    return A


# revision 41
# speedup vs baseline: 1.1045x; 1.1045x over previous
"""Trainium2 Bass kernel for nn_AdaptiveAdjacencyMatrix.

Reference math:
    s[b, i]        = sum_d h[b, i, d] * w[d]
    scores[b,i,j]  = s[b,i] + s[b,j] + bias
    A              = softmax(scores, axis=1)   # over i

Because the softmax is over axis=1 (i), the `s[b,j] + bias` term is constant
along the reduced axis and cancels exactly:
    A[b, i, j] = exp(s[b,i]) / sum_i' exp(s[b,i'])   (independent of j and bias)

So the output is a column-broadcast of softmax(s[b]) — the kernel is purely
memory-bound: write 4*4096*4096*4 = 268 MB of output at HBM speed.

Sharding: 8 cores = (batch b, row-half rh). Each core receives the full
h[b] (rows reordered so its own 2048 rows come first), computes softmax(s)
locally (the softmax sum needs all 4096 rows anyway; row order is
irrelevant to the sum), and writes a [2048, 4096] output shard. No
collectives needed.

Layouts: h is DMA'd with fully-contiguous per-partition descriptors
(partition p holds rows 16p..16p+15 of its half), which makes the on-chip
softmax come out in a (q, r) layout where device output row 128r + q holds
the value for local input row 16q + r. The host unshard undoes that with a
cheap reshape/transpose.
"""

import ml_dtypes
import numpy as np

B, N, D = 4, 4096, 256
NCORES = 8
HALF = N // 2          # 2048 rows written per core
P = 128                # SBUF partitions
RPP = HALF // P        # 16 rows per partition (per half)
DOT_CHUNK = 4          # rows-per-partition per h-load/dot chunk (512 KB)
SUP = 2                # groups per output supertile / DMA (4 MB)

_CACHE = {}


def _build():
    import concourse.mybir as mybir
    import concourse.tile as tile
    from concourse import bacc

    f32 = mybir.dt.float32
    Copy = mybir.ActivationFunctionType.Copy
    nc = bacc.Bacc("TRN2", target_bir_lowering=False, debug=False)

    bf16 = mybir.dt.bfloat16
    h_ext = nc.declare_dram_parameter("h", [N, D], bf16, isOutput=False)
    # w arrives pre-broadcast to 128 partitions (host-side tile of the 1 KB
    # vector) so the dot-product chain only waits on one small DMA.
    w_ext = nc.declare_dram_parameter("wb", [P, D], bf16, isOutput=False)
    out_ext = nc.declare_dram_parameter("out", [HALF, N], f32, isOutput=True)

    # contiguous flat views: partition p holds rows 16p..16p+15 of each half
    h_mine = h_ext[0:HALF, :].rearrange("(p r) d -> p r d", p=P)
    h_oth = h_ext[HALF:N, :].rearrange("(p r) d -> p r d", p=P)
    # [128, r, j] view of out: device out row = 128r + q (host un-permutes)
    out_r = out_ext[:, :].rearrange("(r q) j -> q r j", q=P)

    with tile.TileContext(nc) as tc:
        with (
            tc.tile_pool(name="const", bufs=1) as cpool,
            tc.tile_pool(name="hload", bufs=4) as hpool,
            tc.tile_pool(name="prod", bufs=4) as ppool,
            tc.tile_pool(name="small", bufs=1) as spool,
            tc.tile_pool(name="obuf", bufs=3) as opool,
            tc.tile_pool(name="psum", bufs=1, space="PSUM") as psum_pool,
        ):
            # all-ones [128,128] for the PE cross-partition-sum trick
            ones_k = cpool.tile([P, P], f32)
            nc.vector.memset(ones_k[:, :], 1.0)

            # --- w, pre-broadcast on host; first on the sync ring ---
            w_bc = cpool.tile([P, D], bf16)
            nc.sync.dma_start(out=w_bc[:, :], in_=w_ext[:, :])
            # materialized repeat (real strides, so DVE 2x bf16 mode applies)
            w_rep = cpool.tile([P, DOT_CHUNK, D], bf16)
            nc.vector.tensor_copy(
                w_rep[:, :, :],
                w_bc[:, :].unsqueeze(1).broadcast_to([P, DOT_CHUNK, D]),
            )

            # --- s = h @ w for both halves, [128, 16] each.
            # All h DMAs go on the sync ring only (FIFO: first chunk lands
            # fast; the scalar ring's DMAs would queue behind ACT compute).
            # DVE does all multiplies; reductions split ACT/DVE to balance
            # (ACT accum-reduce costs ~0.69us per row vs DVE 1.22us per
            # 4-row chunk). ---
            s_mine = spool.tile([P, RPP], f32)
            s_oth = spool.tile([P, RPP], f32)
            e_mine = spool.tile([P, RPP], f32)
            rs_m = spool.tile([P, 1], f32)
            rs_o = spool.tile([P, 1], f32)
            tot_psum = psum_pool.tile([P, 1], f32)
            jnk = spool.tile([P, D], f32)
            # reduction assignment per chunk: ACT accum-reduce for these,
            # DVE tensor_reduce for the rest (balances the two engines)
            act_chunks = {0, 1, 3, 5}
            n_chunks = RPP // DOT_CHUNK  # 4 per half
            for half in range(2):
                h_src = h_mine if half == 0 else h_oth
                s_dst = s_mine if half == 0 else s_oth
                for c in range(n_chunks):
                    ci = half * n_chunks + c
                    hch = hpool.tile([P, DOT_CHUNK, D], bf16)
                    nc.sync.dma_start(
                        out=hch[:, :, :],
                        in_=h_src[:, c * DOT_CHUNK : (c + 1) * DOT_CHUNK, :],
                    )
                    prod = ppool.tile([P, DOT_CHUNK, D], bf16)
                    nc.vector.tensor_tensor(
                        out=prod[:, :, :],
                        in0=hch[:, :, :],
                        in1=w_rep[:, :, :],
                        op=mybir.AluOpType.mult,
                    )
                    if ci in act_chunks:
                        for g in range(DOT_CHUNK):
                            gi = c * DOT_CHUNK + g
                            nc.scalar.activation(
                                out=jnk[:, :],
                                in_=prod[:, g, :],
                                func=Copy,
                                accum_out=s_dst[:, gi : gi + 1],
                            )
                    else:
                        nc.vector.tensor_reduce(
                            out=s_dst[:, c * DOT_CHUNK : (c + 1) * DOT_CHUNK],
                            in_=prod[:, :, :],
                            axis=mybir.AxisListType.X,
                            op=mybir.AluOpType.add,
                        )
                if half == 0:
                    # mine half done: exp + its partition-sum can overlap the
                    # other half's dot products
                    nc.scalar.activation(
                        out=e_mine[:, :],
                        in_=s_mine[:, :],
                        func=mybir.ActivationFunctionType.Exp,
                        accum_out=rs_m[:, 0:1],
                    )
                    # route rs through DVE so the PE matmul needs only one
                    # wait (its LdWeights slot fits a single semaphore)
                    rs_m2 = spool.tile([P, 1], f32)
                    nc.vector.tensor_copy(rs_m2[:, 0:1], rs_m[:, 0:1])
                    nc.tensor.matmul(
                        tot_psum[:, 0:1],
                        ones_k[:, 0:P],
                        rs_m2[:, 0:1],
                        start=True,
                        stop=False,
                    )

            # --- finish the softmax sum: exp(other), accumulate its
            # partition-sum into the same PSUM bank, reciprocal, scale ---
            e_oth = spool.tile([P, RPP], f32)
            nc.scalar.activation(
                out=e_oth[:, :],
                in_=s_oth[:, :],
                func=mybir.ActivationFunctionType.Exp,
                accum_out=rs_o[:, 0:1],
            )
            nc.tensor.matmul(
                tot_psum[:, 0:1], ones_k[:, 0:P], rs_o[:, 0:1], start=False, stop=True
            )
            inv = spool.tile([P, 1], f32)
            nc.vector.reciprocal(inv[:, 0:1], tot_psum[:, 0:1])

            # --- broadcast p along columns (stride-0 reads) and stream out.
            # First supertile is a single group so the DMA stream (the
            # rate-limiting resource) starts one broadcast-op earlier. ---
            tiles = [1] + [SUP] * ((RPP - 1) // SUP) + (
                [RPP - 1 - SUP * ((RPP - 1) // SUP)]
                if (RPP - 1) % SUP
                else []
            )
            gi = 0
            for t, width in enumerate(tiles):
                ot = opool.tile([P, SUP * N], f32, tag="ot")
                for g in range(width):
                    # p = e * (1/S) folded into the broadcast multiply
                    col_b = e_mine[:, gi + g : gi + g + 1].broadcast_to([P, N])
                    dst = ot[:, g * N : (g + 1) * N]
                    nc.vector.tensor_scalar_mul(dst, col_b, inv[:, 0:1])
                # alternate the two HWDGE rings for more DMA queue depth
                out_dma_eng = nc.sync if t % 2 == 0 else nc.scalar
                out_dma_eng.dma_start(
                    out=out_r[:, gi : gi + width, :],
                    in_=ot[:, 0 : width * N].rearrange("q (r j) -> q r j", r=width),
                )
                gi += width
    nc.compile()
    return nc


def _get_nc():
    if "nc" not in _CACHE:
        _CACHE["nc"] = _build()
    return _CACHE["nc"]


def _ensure_axon_hooks():
    """bass_utils' trace path imports antenv.axon_hooks, which some images
    lack; provide a no-op stub so tracing degrades instead of crashing."""
    try:
        import antenv.axon_hooks  # noqa: F401
    except ImportError:
        import sys
        import types

        try:
            import antenv
        except ImportError:
            antenv = types.ModuleType("antenv")
            sys.modules["antenv"] = antenv
        m = types.ModuleType("antenv.axon_hooks")
        m._hook = None
        m.set_axon_ntff_profile_hook = lambda h: setattr(m, "_hook", h)
        m.get_axon_ntff_profile_hook = lambda: m._hook
        sys.modules["antenv.axon_hooks"] = m


def run_on_device(h, w, trace=False):
    """Run the SPMD kernel; returns the BassKernelResults."""
    from concourse.bass_utils import run_bass_kernel_spmd

    _ensure_axon_hooks()

    in_maps = []
    for c in range(NCORES):
        b_idx, rh = divmod(c, 2)
        hb = h[b_idx]
        if rh:
            hb = np.concatenate([hb[HALF:], hb[:HALF]], axis=0)
        in_maps.append(
            {
                "h": np.ascontiguousarray(hb.astype(ml_dtypes.bfloat16)),
                "wb": np.ascontiguousarray(
                    np.broadcast_to(w.astype(ml_dtypes.bfloat16), (P, D))
                ),
            }
        )
    res = run_bass_kernel_spmd(
        _get_nc(), in_maps, core_ids=list(range(NCORES)), trace=trace
    )
    return res


def kernel(h, w, b):
    h = np.asarray(h, dtype=np.float32)
    w = np.asarray(w, dtype=np.float32)
    res = run_on_device(h, w)
    A = np.empty((B, N, N), dtype=np.float32)
    for c in range(NCORES):
        b_idx, rh = divmod(c, 2)
        out_c = res.results[c]["out"]
        # device row 128r + q holds the value for local input row 16q + r:
        # undo with reshape/transpose
        unperm = (
            out_c.reshape(RPP, P, N).transpose(1, 0, 2).reshape(HALF, N)
        )
        A[b_idx, rh * HALF : (rh + 1) * HALF, :] = unperm
    return A
